# revision 1
# baseline (speedup 1.0000x reference)
"""Ternary-quantized 3x3 conv (stride 1, pad 1) on 8 trn2 NeuronCores.

Full inputs: X (32,128,56,56) f32, weight (256,128,3,3) f32, Wp/Wn (1,) f32.
Output: (32,256,56,56) f32.

Strategy: data-parallel over batch (4 images per core). Weight quantization
(ternary {-1,0,+1}, exact in fp16) is done host-side and replicated.
Per-core kernel: implicit GEMM — C_IN=128 on SBUF partitions, each image is
cast fp32->fp16 into a zero-padded (58x58) SBUF tile; for each of 2 output-
channel chunks the 9 taps accumulate into 7 PSUM banks (one per 8-row spatial
tile, free dim 448) via 128x128x448 fp16 matmuls with fp32 accumulation.
"""

import sys

sys.path.insert(0, "/opt/trn_rl_repo")

import numpy as np

import bass_rust
import concourse.bass as bass
import concourse.mybir as mybir
from concourse.tile import TileContext
from concourse.bass_utils import run_bass_kernel_spmd

B, C_IN, C_OUT, KS, H, W = 32, 128, 256, 3, 56, 56
THRESHOLD = 0.05
N_CORES = 8
NPC = B // N_CORES  # images per core
HP, WP_ = H + 2, W + 2  # padded spatial
ROWS = 8  # output rows per spatial tile
NT = H // ROWS  # spatial tiles per image (7)
NFREE = ROWS * W  # matmul free dim (448)
OCC = C_OUT // 128  # output channel chunks (2)

# walrus codegen in this container has tight per-instruction sync-wait
# encoding limits (DMA_DIRECT2D: 1, CTRL/Drain: <=2). Hoist excess waits onto
# preceding nop instructions on the same engine (safe: every non-Pool engine
# sequencer is a single strict-FIFO stream).
_MAX_WAITS = {
    "InstDMACopy": 1,
    "InstDrain": 1,
    "InstNop": 1,
    "InstNoOp": 1,
    "InstEventSemaphore": 1,
    "InstSemClear": 1,
}
_DEFAULT_MAX_WAITS = 1


def _split_ctrl_waits(nc, max_waits=None):
    for bbw in nc.main_func.blocks:
        il = bbw.instructions
        i = 0
        while i < len(il):
            ins = il[i]
            si = ins.sync_info
            if si is None or not si.on_wait:
                i += 1
                continue
            limit = _MAX_WAITS.get(type(ins).__name__, _DEFAULT_MAX_WAITS)
            if len(si.on_wait) > limit and str(ins.engine) != "EngineType.Pool":
                max_waits = limit
                waits = list(si.on_wait)
                keep, extra = waits[:max_waits], waits[max_waits:]
                new_insts = []
                for s in range(0, len(extra), max_waits):
                    chunk = extra[s : s + max_waits]
                    nop_ins = nc.engines[ins.engine].nop(nofuse=True).ins
                    for b2 in nc.main_func.blocks:
                        if b2.instructions and b2.instructions[-1] is nop_ins:
                            b2.instructions.pop()
                            break
                    nop_ins.sync_info = bass_rust.SyncInfo(
                        on_wait=chunk, on_update=[]
                    )
                    new_insts.append(nop_ins)
                si.on_wait = keep
                for k, nop_ins in enumerate(new_insts):
                    il.insert(i + k, nop_ins)
                i += len(new_insts)
            i += 1


def _build_nc():
    f32, f16 = mybir.dt.float32, mybir.dt.float16
    nc = bass.Bass()
    x_in = nc.dram_tensor("X", [NPC, C_IN, H, W], f32, kind="ExternalInput")
    w_in = nc.dram_tensor("W", [C_IN, KS * KS, C_OUT], f16, kind="ExternalInput")
    out = nc.dram_tensor("OUT", [NPC, C_OUT, H, W], f32, kind="ExternalOutput")

    with TileContext(nc) as tc:
        with (
            tc.tile_pool(name="wp", bufs=1) as wp,
            tc.tile_pool(name="xs", bufs=2) as xsp,
            tc.tile_pool(name="xq", bufs=2) as xqp,
            tc.tile_pool(name="ps", bufs=8, space="PSUM") as psp,
            tc.tile_pool(name="ob", bufs=8) as obp,
        ):
            wt = wp.tile([C_IN, KS * KS, C_OUT], f16)

            # PE warm-up: dummy matmuls on scratch SBUF keep TensorE busy
            # through the input-load phase so HAM is at K=8/8 (2.4 GHz) when
            # the real matmuls start (saves the ~2us cold ramp, and the
            # ~3.4us idle window never elapses before real work arrives).
            warm_sb = wp.tile([C_IN, 384], f16, name="warm_sb", tag="warm_sb")
            nc.gpsimd.memset(warm_sb[:], 0.0)
            warm_ps = psp.tile([128, 256], f32, name="warm_ps", tag="warm", bufs=1)
            for _ in range(20):
                nc.tensor.matmul(
                    warm_ps[:], warm_sb[:, 0:128], warm_sb[:, 128:384], start=True, stop=True
                )

            def load_chunk(xs, xq, n, r0, nrows):
                nc.sync.dma_start(
                    out=xs[:, r0 : r0 + nrows, :], in_=x_in[n, :, r0 : r0 + nrows, :]
                )
                nc.vector.tensor_copy(
                    xq[:, r0 + 1 : r0 + nrows + 1, 1 : WP_ - 1],
                    xs[:, r0 : r0 + nrows, :],
                )

            CH_STEADY = [(0, 14), (14, 14), (28, 14), (42, 14)]

            xs0 = xsp.tile([C_IN, H, W], f32, name="xs_0", tag="xs")
            xq0 = xqp.tile([C_IN, HP, WP_], f16, name="xq_0", tag="xq")
            # image-0 borders first in DVE program order, ahead of the casts
            nc.vector.memset(xq0[:, 0, :], 0.0)
            nc.vector.memset(xq0[:, HP - 1, :], 0.0)
            nc.vector.memset(xq0[:, 1 : HP - 1, 0], 0.0)
            nc.vector.memset(xq0[:, 1 : HP - 1, WP_ - 1], 0.0)
            # priority order: first X chunk, tap-0 weights, remaining weights,
            # remaining X chunks — first matmul group needs only W[:,0] +
            # X rows 0..8, and HBM sustains the stream from there
            load_chunk(xs0, xq0, 0, 0, 7)
            load_chunk(xs0, xq0, 0, 7, 7)
            nc.sync.dma_start(out=wt[:, 0:3, :], in_=w_in[:, 0:3, :])
            load_chunk(xs0, xq0, 0, 14, 7)
            load_chunk(xs0, xq0, 0, 21, 7)
            nc.sync.dma_start(out=wt[:, 3:9, :], in_=w_in[:, 3:9, :])
            load_chunk(xs0, xq0, 0, 28, 14)
            load_chunk(xs0, xq0, 0, 42, 14)

            for n in range(NPC):
                if n == 0:
                    xs, xq = xs0, xq0
                else:
                    xs = xsp.tile([C_IN, H, W], f32, name=f"xs_{n}", tag="xs")
                    xq = xqp.tile([C_IN, HP, WP_], f16, name=f"xq_{n}", tag="xq")
                    nc.vector.memset(xq[:, 0, :], 0.0)
                    nc.vector.memset(xq[:, HP - 1, :], 0.0)
                    nc.vector.memset(xq[:, 1 : HP - 1, 0], 0.0)
                    nc.vector.memset(xq[:, 1 : HP - 1, WP_ - 1], 0.0)
                    for r0, nr in CH_STEADY:
                        load_chunk(xs, xq, n, r0, nr)

                for oc in range(OCC):
                    for s in range(NT):
                        ps = psp.tile(
                            [128, NFREE], f32, tag="ps", name=f"ps_{n}_{oc}_{s}", bufs=7
                        )
                        for t in range(KS * KS):
                            kh, kw = divmod(t, KS)
                            lhsT = wt[:, t, oc * 128 : (oc + 1) * 128]
                            rhs = xq[:, s * ROWS + kh : s * ROWS + ROWS + kh, kw : kw + W]
                            nc.tensor.matmul(
                                ps[:],
                                lhsT,
                                rhs,
                                start=(t == 0),
                                stop=(t == KS * KS - 1),
                            )
                        ob = obp.tile([128, NFREE], f32)
                        last = n == NPC - 1 and oc == OCC - 1 and s == NT - 1
                        if not last:
                            nc.any.tensor_copy(ob[:], ps[:])
                            nc.sync.dma_start(
                                out=out[n, oc * 128 : (oc + 1) * 128, s * ROWS : (s + 1) * ROWS, :],
                                in_=ob[:],
                            )
                        else:
                            # final tile: split evac across DVE+ACT and the
                            # store across two DMA queues to shorten the tail
                            hf = NFREE // 2
                            nc.vector.tensor_copy(ob[:, 0:hf], ps[:, 0:hf])
                            nc.scalar.copy(ob[:, hf:NFREE], ps[:, hf:NFREE])
                            r0 = s * ROWS
                            nc.sync.dma_start(
                                out=out[n, oc * 128 :, r0 : r0 + ROWS // 2, :],
                                in_=ob[:, 0:hf],
                            )
                            nc.scalar.dma_start(
                                out=out[n, oc * 128 :, r0 + ROWS // 2 : r0 + ROWS, :],
                                in_=ob[:, hf:NFREE],
                            )
    _split_ctrl_waits(nc)
    return nc


_NC_CACHE = None


def _ensure_axon_hooks_stub():
    """bass_utils imports antenv.axon_hooks when tracing is requested (e.g. a
    BASS_TRACE env var); the agent image's antenv lacks that module. Provide a
    no-op hook module so tracing degrades gracefully instead of crashing."""
    try:
        import antenv.axon_hooks  # noqa: F401
    except ImportError:
        import types

        mod = types.ModuleType("antenv.axon_hooks")
        mod.get_axon_ntff_profile_hook = lambda: None
        mod.set_axon_ntff_profile_hook = lambda h: None
        sys.modules["antenv.axon_hooks"] = mod


def _quantize(weight):
    """Exact replica of the reference's ternary quantization, in numpy f32."""
    t = np.float32(THRESHOLD)
    nw = (weight / np.max(np.abs(weight))).astype(np.float32)
    mask = np.where((nw > -t) & (nw <= t), np.float32(0.0), nw)
    mask = np.where(mask > t, np.float32(1.0), mask)
    mask = np.where(mask < -t, np.float32(-1.0), mask)
    qw = np.where(mask == np.float32(-1.0), np.float32(-1.0), mask)
    return qw.astype(np.float32)


def kernel(X, weight, Wp, Wn):
    global _NC_CACHE
    X = np.ascontiguousarray(np.asarray(X, dtype=np.float32))
    weight = np.asarray(weight, dtype=np.float32)
    Wn_val = np.float32(np.asarray(Wn).reshape(-1)[0])

    qw = _quantize(weight)
    # reference maps -1 -> Wn (broadcast); replicate that faithfully
    qw = np.where(qw == np.float32(-1.0), Wn_val, qw).astype(np.float32)
    # (C_OUT, C_IN, 3, 3) -> (C_IN, 9, C_OUT), fp16 (ternary values exact)
    wq = np.ascontiguousarray(
        qw.transpose(1, 2, 3, 0).reshape(C_IN, KS * KS, C_OUT)
    ).astype(np.float16)

    _ensure_axon_hooks_stub()
    if _NC_CACHE is None:
        _NC_CACHE = _build_nc()
    nc = _NC_CACHE

    in_maps = [
        {"X": X[c * NPC : (c + 1) * NPC], "W": wq} for c in range(N_CORES)
    ]
    res = run_bass_kernel_spmd(nc, in_maps, core_ids=list(range(N_CORES)))
    return np.concatenate([res.results[c]["OUT"] for c in range(N_CORES)], axis=0)

